# revision 17
# baseline (speedup 1.0000x reference)
"""MAGC (multi-header attention global context) pooling kernel for Trainium2.

Math (per sample, reference.py):
    xh[g, n, :]   = x[n, g*64:(g+1)*64]                (g=8 headers, n=H*W)
    logits[g, n]  = (xh[g, n, :] . w_mask + b_mask) / 8
    attn          = softmax_n(logits)
    ctx[g, :]     = sum_n attn[g, n] * xh[g, n, :]     -> ctx [C]
    t             = relu(LN(ctx @ w1 + b1)) @ w2 + b2
    out           = x + t  (broadcast over n)

Sharding: pure data parallel, 16 samples -> 8 cores x 2 samples.

v3 dataflow (per core): x is cast-DMA'd (SWDGE, f32->f16 inline,
contiguous 12KB/partition reads) into a persistent fp16 SBUF residency
covering BOTH samples, so sample 1's input stream overlaps sample 0's
residual/store phase.  Per chunk (6 tiles): DVE mul + 2x pair-add +
segmented reduce -> ACT exp -> PE ctx matmuls col-tiled 4x (strips at
PSUM partitions 0/32/64/96 run concurrently).  Per sample: softmax
denom; ctx extract masks all strips in one DVE op, PE-transposes
[128,128] channel blocks and row-reduces, with the 1/S softmax scale
applied as a per-partition scalar built by a tiny PE matmul; then the
MLP.  Residual x + t is a DVE fp16 add against a broadcast t tile;
output is stored fp16 (tolerance 2e-2) and upcast on the host.
"""

import sys

import numpy as np

if "/opt/trn_rl_repo" not in sys.path:
    sys.path.insert(0, "/opt/trn_rl_repo")

B, H, W, C = 16, 48, 160, 512
G = 8                 # attention headers
SHI = C // G          # 64 channels per header
N = H * W             # 7680 spatial positions per sample
P = 128               # SBUF partitions
NT = N // P           # 60 [128, C] tiles per sample
NCORES = 8
BPC = B // NCORES     # samples per core
NB = C // P           # 4 channel blocks of 128
LN_EPS = 1e-3
KCH = 6               # [128, C] tiles per processing chunk (1.5 MB reads)
NCHK = NT // KCH      # 10 chunks per sample
NSTRIP = 4            # ctx col-tiling strips (PE 32-col groups)
# per-sample chunk plans: (t0, kch, hw_bootstrap); sample 0 leads with
# two 3-tile chunks so the first DVE work starts ~4us earlier
PLAN0 = [(0, 3, False), (3, 3, False)] + [
    (6 + i * KCH, KCH, False) for i in range((NT - 6) // KCH)
]
PLAN1 = [(i * KCH, KCH, False) for i in range(NT // KCH)]
PLANS = {0: PLAN0, 1: PLAN1}


def build_nc():
    import concourse.tile as tile
    from concourse import bacc, mybir

    f32 = mybir.dt.float32
    f16 = mybir.dt.float16
    AX = mybir.AxisListType.X
    MUL = mybir.AluOpType.mult
    SUB = mybir.AluOpType.subtract
    AF = mybir.ActivationFunctionType

    nc = bacc.Bacc()

    x_d = nc.dram_tensor("x", [BPC, H, W, C], f32, kind="ExternalInput")
    wrep_d = nc.dram_tensor("w_rep", [P, KCH, C], f16, kind="ExternalInput")
    bb_d = nc.dram_tensor("b_bias", [P, 1], f32, kind="ExternalInput")
    w1_d = nc.dram_tensor("w1p", [P, NB, C], f16, kind="ExternalInput")
    w2_d = nc.dram_tensor("w2p", [P, NB, C], f16, kind="ExternalInput")
    b1_d = nc.dram_tensor("b1r", [1, C], f16, kind="ExternalInput")
    b2_d = nc.dram_tensor("b2r", [1, C], f16, kind="ExternalInput")
    gm_d = nc.dram_tensor("gammar", [1, C], f32, kind="ExternalInput")
    bt_d = nc.dram_tensor("betar", [1, C], f32, kind="ExternalInput")
    ms_d = nc.dram_tensor("mask_sel4", [P, C], f32, kind="ExternalInput")
    id_d = nc.dram_tensor("ident", [P, P], f32, kind="ExternalInput")
    l16_d = nc.dram_tensor("l16", [G, P], f32, kind="ExternalInput")
    in_d = nc.dram_tensor("indn", [G, NB], f32, kind="ExternalInput")
    on_d = nc.dram_tensor("ones_row", [1, P], f32, kind="ExternalInput")
    onh_d = nc.dram_tensor("ones_h", [1, P], f16, kind="ExternalInput")
    out_d = nc.dram_tensor("out", [BPC, H, W, C], f16, kind="ExternalOutput")

    xf = x_d.rearrange("b h w c -> (b h w) c")
    of = out_d.rearrange("b h w c -> (b h w) c")

    with tile.TileContext(nc) as tc:
        with (
            tc.tile_pool(name="consts", bufs=1) as consts,
            tc.tile_pool(name="xhp", bufs=2 * NCHK) as xhp,
            tc.tile_pool(name="xwp", bufs=2) as xwp,
            tc.tile_pool(name="xsp", bufs=2) as xsp,
            tc.tile_pool(name="lgp", bufs=2) as lgp,
            tc.tile_pool(name="smp", bufs=1) as smp,
            tc.tile_pool(name="trp", bufs=2) as trp,
            tc.tile_pool(name="ctxps", bufs=2, space="PSUM") as ctxps,
            tc.tile_pool(name="rps", bufs=2, space="PSUM") as rps,
            tc.tile_pool(name="mps", bufs=2, space="PSUM") as mps,
        ):
            # issue the first input chunks before anything else so the
            # SWDGE pipeline (and its one-time ucode load) starts at t=0
            early_xh = []
            for ck0 in range(2):
                et0, ekch, _ = PLAN0[ck0]
                xh0 = xhp.tile([P, ekch, C], f16, tag="xh")
                rows0 = xf[et0 * P : (et0 + ekch) * P, :]
                nc.gpsimd.dma_start(
                    xh0, rows0.rearrange("(p k) c -> p k c", k=ekch)
                )
                early_xh.append(xh0)

            w_rep = consts.tile([P, KCH, C], f16)
            nc.sync.dma_start(w_rep, wrep_d[:, :, :])
            bb = consts.tile([P, 1], f32)
            nc.sync.dma_start(bb, bb_d[:, :])
            w1s = consts.tile([P, NB, C], f16)
            nc.sync.dma_start(w1s, w1_d[:, :, :])
            w2s = consts.tile([P, NB, C], f16)
            nc.sync.dma_start(w2s, w2_d[:, :, :])
            b1s = consts.tile([1, C], f16)
            nc.sync.dma_start(b1s, b1_d[:, :])
            b2s = consts.tile([1, C], f16)
            nc.sync.dma_start(b2s, b2_d[:, :])
            gms = consts.tile([1, C], f32)
            nc.sync.dma_start(gms, gm_d[:, :])
            bts = consts.tile([1, C], f32)
            nc.sync.dma_start(bts, bt_d[:, :])
            msel4 = consts.tile([P, C], f32)
            nc.sync.dma_start(msel4, ms_d[:, :])
            ident = consts.tile([P, P], f32)
            nc.sync.dma_start(ident, id_d[:, :])
            l16 = consts.tile([G, P], f32)
            nc.sync.dma_start(l16, l16_d[:, :])
            indn = consts.tile([G, NB], f32)
            nc.sync.dma_start(indn, in_d[:, :])
            ones_r = consts.tile([1, P], f32)
            nc.sync.dma_start(ones_r, on_d[:, :])
            ones_h = consts.tile([1, P], f16)
            nc.sync.dma_start(ones_h, onh_d[:, :])
            eps_t = consts.tile([1, 1], f32)
            nc.vector.memset(eps_t, LN_EPS)

            # Sem-absorption ops: walrus allows very few sync waits per
            # compute instruction, so let each engine observe the const-load
            # DMA sems via tiny reads up front, keeping the hot-loop
            # instructions at <=1 wait each.
            cs4r = consts.tile([P, C], f32)
            nc.vector.memset(cs4r, 0.0)
            ab_ac = smp.tile([1, 1], f32, tag="ab_ac")
            nc.scalar.copy(ab_ac, bb[0:1, 0:1])
            nc.scalar.copy(ab_ac, eps_t[0:1, 0:1])
            ab_dv = smp.tile([1, 1], f32, tag="ab_dv")
            nc.vector.tensor_copy(ab_dv, msel4[0:1, 0:1])
            ab_te = mps.tile([1, 1], f32, tag="m")
            nc.tensor.matmul(ab_te, ones_h[:, 0:1], ones_h[:, 0:1],
                             start=True, stop=True)

            states = {}

            def emit_attn_chunk(s, ck):
                if ck == 0:
                    logits = lgp.tile([P, NT, G], f32, tag="logits")
                    esb = lgp.tile([P, NT, G], f16, tag="esb")
                    ctx_ps = ctxps.tile([P, C], f32, tag="ctx")
                    states[s] = (logits, esb, ctx_ps, [])
                logits, esb, ctx_ps, xhtiles = states[s]
                base = s * N
                t0, kch, hw = PLANS[s][ck]
                if s == 0 and ck < len(early_xh):
                    xh = early_xh[ck]
                else:
                    xh = xhp.tile([P, kch, C], f16, tag="xh")
                    rows = xf[base + t0 * P : base + (t0 + kch) * P, :]
                    # SWDGE cast-DMA: f32 HBM -> f16 SBUF at line rate;
                    # each partition line is one contiguous read.
                    nc.gpsimd.dma_start(
                        xh, rows.rearrange("(p k) c -> p k c", k=kch)
                    )
                xhtiles.append(xh)

                xw = xwp.tile([P, kch, C], f16, tag="xw")
                nc.vector.tensor_mul(xw, xh, w_rep[:, :kch, :])
                xwv = xw.rearrange("p k (g s) -> p k g s", s=SHI)
                x1 = xsp.tile([P, kch, G, SHI // 2], f16, tag="x1")
                nc.vector.tensor_add(
                    x1, xwv[:, :, :, : SHI // 2], xwv[:, :, :, SHI // 2 :]
                )
                x2 = xsp.tile([P, kch, G, SHI // 4], f16, tag="x2")
                nc.vector.tensor_add(
                    x2, x1[:, :, :, : SHI // 4], x1[:, :, :, SHI // 4 :]
                )
                nc.vector.reduce_sum(logits[:, t0 : t0 + kch, :], x2, AX)

                # E = exp((dot + b_mask) / 8); |logits| < ~1 so no
                # max-subtraction is needed for stability.
                nc.scalar.activation(
                    esb[:, t0 : t0 + kch, :],
                    logits[:, t0 : t0 + kch, :],
                    AF.Exp,
                    bias=bb[:, 0:1],
                    scale=0.125,
                )

                # ctx matmuls, col-tiled: strip j = t % 4 accumulates
                # into ctx_ps[32j:32j+8, :]; the 4 strips use distinct
                # 32-col PE groups and run concurrently.
                for t in range(t0, t0 + kch):
                    j = t % NSTRIP
                    nc.tensor.matmul(
                        ctx_ps[32 * j : 32 * j + G, :],
                        esb[:, t, :],
                        xh[:, t - t0, :],
                        start=(t == j),
                        stop=(t >= NT - NSTRIP),
                        tile_position=(0, 32 * j),
                        skip_group_check=True,
                    )

            def tail_chain(s, res):
                """softmax denom + ctx extract + MLP + t broadcast, as a
                generator so its steps can be woven between other phases'
                chunks (each step's cross-engine deps then resolve during
                the neighbouring chunk's work instead of stalling DVE)."""
                logits, esb, ctx_ps, xhtiles = states[s]

                # ---- softmax denominator: S[g] = sum_n E[n, g]
                sp = smp.tile([P, G], f32, tag="sp")
                nc.vector.reduce_sum(sp, esb.rearrange("p t g -> p g t"), AX)
                for j in range(NSTRIP):
                    nc.scalar.copy(
                        cs4r[32 * j : 32 * j + G, :],
                        ctx_ps[32 * j : 32 * j + G, :],
                    )
                yield
                spt = mps.tile([G, P], f32, tag="m")
                nc.tensor.transpose(spt, sp, ident)
                ssum = smp.tile([G, 1], f32, tag="ssum")
                nc.vector.reduce_sum(ssum, spt, AX)
                sinv = smp.tile([G, 1], f32, tag="sinv")
                nc.vector.reciprocal(sinv, ssum)
                yield
                # ---- 1/S as a per-partition column: sinv_pn[p, cb] =
                # sinv[2*cb + p//64], built as l16.T @ (indn * sinv).
                rsc = smp.tile([G, NB], f32, tag="rsc")
                nc.vector.tensor_scalar(
                    out=rsc, in0=indn, scalar1=sinv, scalar2=None, op0=MUL
                )
                spn_ps = mps.tile([P, NB], f32, tag="m")
                nc.tensor.matmul(spn_ps, l16, rsc, start=True, stop=True)
                sinv_pn = smp.tile([P, NB], f32, tag="sinv_pn")
                nc.vector.tensor_copy(sinv_pn, spn_ps)
                yield
                # ---- ctx extract: strips were copied out of PSUM into the
                # zeroed cs4r; mask all strips in one DVE op, then transpose
                # each 128-channel block and reduce (non-strip rows zero).
                cs4 = smp.tile([P, C], f32, tag="cs4")
                nc.vector.tensor_mul(cs4, cs4r, msel4)
                yield
                ctxt = smp.tile([P, NB], f32, tag="ctxt")
                for cb in range(NB):
                    tp4 = mps.tile([P, P], f32, tag="m")
                    nc.tensor.transpose(
                        tp4, cs4[:, cb * P : (cb + 1) * P], ident
                    )
                    nc.vector.reduce_sum(ctxt[:, cb : cb + 1], tp4, AX)
                    if cb == 1:
                        yield
                ctxs16 = smp.tile([P, NB], f16, tag="ctxs16")
                nc.vector.tensor_mul(ctxs16, ctxt, sinv_pn)
                yield
                # ---- h = ctx @ w1 + b1
                h_ps = mps.tile([1, C], f32, tag="m")
                for j in range(NB):
                    nc.tensor.matmul(
                        h_ps,
                        ctxs16[:, j : j + 1],
                        w1s[:, j, :],
                        start=(j == 0), stop=False,
                    )
                nc.tensor.matmul(
                    h_ps, ones_h[:, 0:1], b1s, start=False, stop=True,
                )
                yield
                # ---- LayerNorm over C, then ReLU
                musum = smp.tile([1, 1], f32, tag="musum")
                nc.vector.reduce_sum(musum, h_ps, AX)
                mu = smp.tile([1, 1], f32, tag="mu")
                nc.vector.tensor_scalar_mul(mu, musum, 1.0 / C)
                hc = smp.tile([1, C], f32, tag="hc")
                nc.vector.tensor_scalar(
                    out=hc, in0=h_ps, scalar1=mu, scalar2=None, op0=SUB
                )
                sq = smp.tile([1, C], f32, tag="sq")
                varsum = smp.tile([1, 1], f32, tag="varsum")
                nc.scalar.activation(sq, hc, AF.Square, accum_out=varsum)
                yield
                std = smp.tile([1, 1], f32, tag="std")
                nc.scalar.activation(
                    std, varsum, AF.Sqrt, bias=eps_t[:, 0:1], scale=1.0 / C
                )
                rstd = smp.tile([1, 1], f32, tag="rstd")
                nc.vector.reciprocal(rstd, std)
                hn = smp.tile([1, C], f32, tag="hn")
                nc.vector.scalar_tensor_tensor(
                    out=hn, in0=hc, scalar=rstd, in1=gms, op0=MUL, op1=MUL
                )
                hb = smp.tile([1, C], f32, tag="hb")
                nc.vector.tensor_add(hb, hn, bts)
                rl = smp.tile([1, C], f32, tag="rl")
                nc.scalar.activation(rl, hb, AF.Relu)
                yield
                # ---- t = relu_h @ w2 + b2 (transpose relu_h to [128, 4])
                rt_ps = mps.tile([P, NB], f32, tag="m")
                for j in range(NB):
                    nc.tensor.transpose(
                        rt_ps[:, j : j + 1],
                        rl[:, j * P : (j + 1) * P],
                        ones_r[:, 0:1],
                    )
                rts = smp.tile([P, NB], f16, tag="rts")
                nc.vector.tensor_copy(rts, rt_ps)
                yield
                t_ps = mps.tile([1, C], f32, tag="m")
                for j in range(NB):
                    nc.tensor.matmul(
                        t_ps,
                        rts[:, j : j + 1],
                        w2s[:, j, :],
                        start=(j == 0), stop=False,
                    )
                nc.tensor.matmul(
                    t_ps, ones_h[:, 0:1], b2s, start=False, stop=True,
                )
                tsb16 = smp.tile([1, C], f16, tag="tsb16")
                nc.scalar.copy(tsb16, t_ps)
                yield
                # ---- broadcast t to all partitions, fp16
                trep_ps = rps.tile([P, C], f32, tag="rp")
                nc.tensor.matmul(trep_ps, ones_h, tsb16, start=True, stop=True)
                trep = trp.tile([P, KCH // 2, C], f16, tag="trep")
                for k in range(KCH // 2):
                    nc.scalar.copy(trep[:, k, :], trep_ps)
                res["trep"] = trep

            def emit_resid_chunk(s, ck, trep):
                # residual add out = x + t, fp16, in place on xh
                xhtiles = states[s][3]
                xh = xhtiles[ck]
                base = s * N
                t0, kch, _ = PLANS[s][ck]
                lo = 0
                while lo < kch:
                    hi = min(lo + KCH // 2, kch)
                    nc.vector.tensor_add(
                        xh[:, lo:hi, :], xh[:, lo:hi, :], trep[:, : hi - lo, :]
                    )
                    lo = hi
                rows = of[base + t0 * P : base + (t0 + kch) * P, :]
                nc.sync.dma_start(
                    rows.rearrange("(p k) c -> p k c", k=kch), xh
                )

            def drive(gen):
                try:
                    next(gen)
                    return True
                except StopIteration:
                    return False

            # ---- interleaved emission: sample 0's tail chain is woven
            # into sample 1's attention chunks; sample 1's tail chain into
            # sample 0's residual/store chunks.  Each engine's stream then
            # alternates chain steps with bulk work, so the chain's
            # cross-engine latency hides behind the bulk ops.
            for ck in range(len(PLAN0)):
                emit_attn_chunk(0, ck)
            res0, res1 = {}, {}
            # chain0 woven into sample 1's first attention chunks; then
            # sample 0's residual adds+stores interleave with the rest of
            # sample 1's attention so the output stream starts early.
            g0 = tail_chain(0, res0)
            for ck in range(len(PLAN1)):
                emit_attn_chunk(1, ck)
                drive(g0)
                drive(g0)
            while drive(g0):
                pass
            g1 = tail_chain(1, res1)
            for ck in range(len(PLAN0)):
                emit_resid_chunk(0, ck, res0["trep"])
                drive(g1)
                drive(g1)
            while drive(g1):
                pass
            for ck in range(len(PLAN1)):
                emit_resid_chunk(1, ck, res1["trep"])

    nc.finalize()
    return nc


def _prep_shared(inputs):
    w_mask = np.asarray(inputs["w_mask"], np.float32).reshape(SHI)
    b_mask = np.asarray(inputs["b_mask"], np.float32).reshape(1)
    w1 = np.asarray(inputs["w1"], np.float32)
    w2 = np.asarray(inputs["w2"], np.float32)

    # strip-expanded header mask: rows {32*j + g} hold header g's channel
    # mask (ctx strip j lands on PSUM partitions 32*j..32*j+7)
    msel4 = np.zeros((P, C), np.float32)
    gsel = ((np.arange(C)[None, :] // SHI) == np.arange(G)[:, None]).astype(
        np.float32
    )
    for j in range(NSTRIP):
        msel4[32 * j : 32 * j + G, :] = gsel
    # l16[g, p] = (g%2 == p//64); indn[g, cb] = (g//2 == cb):
    # (l16.T @ (indn * sinv))[p, cb] = sinv[2*cb + p//64]
    l16 = (np.arange(G)[:, None] % 2 == np.arange(P)[None, :] // 64).astype(
        np.float32
    )
    indn = (np.arange(G)[:, None] // 2 == np.arange(NB)[None, :]).astype(
        np.float32
    )

    shared = {
        "w_rep": np.broadcast_to(np.tile(w_mask, G), (P, KCH, C)).astype(
            np.float16
        ),
        "b_bias": np.full((P, 1), b_mask[0] * 0.125, np.float32),
        "w1p": np.ascontiguousarray(
            w1.reshape(NB, P, C).transpose(1, 0, 2)
        ).astype(np.float16),
        "w2p": np.ascontiguousarray(
            w2.reshape(NB, P, C).transpose(1, 0, 2)
        ).astype(np.float16),
        "b1r": np.asarray(inputs["b1"], np.float16).reshape(1, C),
        "b2r": np.asarray(inputs["b2"], np.float16).reshape(1, C),
        "gammar": np.asarray(inputs["gamma"], np.float32).reshape(1, C),
        "betar": np.asarray(inputs["beta"], np.float32).reshape(1, C),
        "mask_sel4": msel4,
        "ident": np.eye(P, dtype=np.float32),
        "l16": l16,
        "indn": indn,
        "ones_row": np.ones((1, P), np.float32),
        "ones_h": np.ones((1, P), np.float16),
    }
    return shared


def make_in_maps(inputs):
    x = np.asarray(inputs["x"], np.float32)
    shared = _prep_shared(inputs)
    in_maps = []
    for i in range(NCORES):
        m = dict(shared)
        m["x"] = np.ascontiguousarray(x[i * BPC : (i + 1) * BPC])
        in_maps.append(m)
    return in_maps


def _axon_device_reset():
    """Clear any wedged NRT exec-unit state left by a previous session."""
    try:
        import ctypes

        import jax

        jax.devices()
        lib = ctypes.CDLL("/opt/axon/libaxon_pjrt.so")
        lib.axon_reset.restype = ctypes.c_int64
        lib.axon_reset()
    except Exception:
        pass


def kernel(**inputs):
    from concourse.bass_utils import run_bass_kernel_spmd

    _axon_device_reset()
    nc = build_nc()
    in_maps = make_in_maps(inputs)
    res = run_bass_kernel_spmd(nc, in_maps, list(range(NCORES)))
    out = np.concatenate([r["out"] for r in res.results], axis=0)
    return out.astype(np.float32)


# revision 19
# speedup vs baseline: 1.0950x; 1.0950x over previous
"""MAGC (multi-header attention global context) pooling kernel for Trainium2.

Math (per sample, reference.py):
    xh[g, n, :]   = x[n, g*64:(g+1)*64]                (g=8 headers, n=H*W)
    logits[g, n]  = (xh[g, n, :] . w_mask + b_mask) / 8
    attn          = softmax_n(logits)
    ctx[g, :]     = sum_n attn[g, n] * xh[g, n, :]     -> ctx [C]
    t             = relu(LN(ctx @ w1 + b1)) @ w2 + b2
    out           = x + t  (broadcast over n)

Sharding: pure data parallel, 16 samples -> 8 cores x 2 samples.

v3 dataflow (per core): x is cast-DMA'd (SWDGE, f32->f16 inline,
contiguous 12KB/partition reads) into a persistent fp16 SBUF residency
covering BOTH samples, so sample 1's input stream overlaps sample 0's
residual/store phase.  Per chunk (6 tiles): DVE mul + 2x pair-add +
segmented reduce -> ACT exp -> PE ctx matmuls col-tiled 4x (strips at
PSUM partitions 0/32/64/96 run concurrently).  Per sample: softmax
denom; ctx extract masks all strips in one DVE op, PE-transposes
[128,128] channel blocks and row-reduces, with the 1/S softmax scale
applied as a per-partition scalar built by a tiny PE matmul; then the
MLP.  Residual x + t is a DVE fp16 add against a broadcast t tile;
output is stored fp16 (tolerance 2e-2) and upcast on the host.
"""

import sys

import numpy as np

if "/opt/trn_rl_repo" not in sys.path:
    sys.path.insert(0, "/opt/trn_rl_repo")

B, H, W, C = 16, 48, 160, 512
G = 8                 # attention headers
SHI = C // G          # 64 channels per header
N = H * W             # 7680 spatial positions per sample
P = 128               # SBUF partitions
NT = N // P           # 60 [128, C] tiles per sample
NCORES = 8
BPC = B // NCORES     # samples per core
NB = C // P           # 4 channel blocks of 128
LN_EPS = 1e-3
KCH = 6               # [128, C] tiles per processing chunk (1.5 MB reads)
NCHK = NT // KCH      # 10 chunks per sample
NSTRIP = 4            # ctx col-tiling strips (PE 32-col groups)
# per-sample chunk plans: (t0, kch, hw_bootstrap)
PLAN0 = [(i * KCH, KCH, False) for i in range(NT // KCH)]
PLAN1 = [(i * KCH, KCH, False) for i in range(NT // KCH)]
PLANS = {0: PLAN0, 1: PLAN1}


def build_nc():
    import concourse.tile as tile
    from concourse import bacc, mybir

    f32 = mybir.dt.float32
    f16 = mybir.dt.float16
    AX = mybir.AxisListType.X
    MUL = mybir.AluOpType.mult
    SUB = mybir.AluOpType.subtract
    AF = mybir.ActivationFunctionType

    nc = bacc.Bacc()

    x_d = nc.dram_tensor("x", [BPC, H, W, C], f32, kind="ExternalInput")
    wrep_d = nc.dram_tensor("w_rep", [P, KCH, C], f16, kind="ExternalInput")
    bb_d = nc.dram_tensor("b_bias", [P, 1], f32, kind="ExternalInput")
    w1_d = nc.dram_tensor("w1p", [P, NB, C], f16, kind="ExternalInput")
    w2_d = nc.dram_tensor("w2p", [P, NB, C], f16, kind="ExternalInput")
    b1_d = nc.dram_tensor("b1r", [1, C], f16, kind="ExternalInput")
    b2_d = nc.dram_tensor("b2r", [1, C], f16, kind="ExternalInput")
    gm_d = nc.dram_tensor("gammar", [1, C], f32, kind="ExternalInput")
    bt_d = nc.dram_tensor("betar", [1, C], f32, kind="ExternalInput")
    ms_d = nc.dram_tensor("mask_sel4", [P, C], f32, kind="ExternalInput")
    id_d = nc.dram_tensor("ident", [P, P], f32, kind="ExternalInput")
    l16_d = nc.dram_tensor("l16", [G, P], f32, kind="ExternalInput")
    in_d = nc.dram_tensor("indn", [G, NB], f32, kind="ExternalInput")
    on_d = nc.dram_tensor("ones_row", [1, P], f32, kind="ExternalInput")
    onh_d = nc.dram_tensor("ones_h", [1, P], f16, kind="ExternalInput")
    out_d = nc.dram_tensor("out", [BPC, H, W, C], f16, kind="ExternalOutput")

    xf = x_d.rearrange("b h w c -> (b h w) c")
    of = out_d.rearrange("b h w c -> (b h w) c")

    with tile.TileContext(nc) as tc:
        with (
            tc.tile_pool(name="consts", bufs=1) as consts,
            tc.tile_pool(name="xhp", bufs=2 * NCHK) as xhp,
            tc.tile_pool(name="xwp", bufs=2) as xwp,
            tc.tile_pool(name="xsp", bufs=2) as xsp,
            tc.tile_pool(name="lgp", bufs=2) as lgp,
            tc.tile_pool(name="smp", bufs=1) as smp,
            tc.tile_pool(name="trp", bufs=2) as trp,
            tc.tile_pool(name="ctxps", bufs=2, space="PSUM") as ctxps,
            tc.tile_pool(name="rps", bufs=2, space="PSUM") as rps,
            tc.tile_pool(name="mps", bufs=2, space="PSUM") as mps,
        ):
            # issue the first input chunks before anything else so the
            # SWDGE pipeline (and its one-time ucode load) starts at t=0
            early_xh = []
            for ck0 in range(2):
                et0, ekch, _ = PLAN0[ck0]
                xh0 = xhp.tile([P, ekch, C], f16, tag="xh")
                rows0 = xf[et0 * P : (et0 + ekch) * P, :]
                nc.gpsimd.dma_start(
                    xh0, rows0.rearrange("(p k) c -> p k c", k=ekch)
                )
                early_xh.append(xh0)

            w_rep = consts.tile([P, KCH, C], f16)
            nc.sync.dma_start(w_rep, wrep_d[:, :, :])
            bb = consts.tile([P, 1], f32)
            nc.sync.dma_start(bb, bb_d[:, :])
            w1s = consts.tile([P, NB, C], f16)
            nc.sync.dma_start(w1s, w1_d[:, :, :])
            w2s = consts.tile([P, NB, C], f16)
            nc.sync.dma_start(w2s, w2_d[:, :, :])
            b1s = consts.tile([1, C], f16)
            nc.sync.dma_start(b1s, b1_d[:, :])
            b2s = consts.tile([1, C], f16)
            nc.sync.dma_start(b2s, b2_d[:, :])
            gms = consts.tile([1, C], f32)
            nc.sync.dma_start(gms, gm_d[:, :])
            bts = consts.tile([1, C], f32)
            nc.sync.dma_start(bts, bt_d[:, :])
            msel4 = consts.tile([P, C], f32)
            nc.sync.dma_start(msel4, ms_d[:, :])
            ident = consts.tile([P, P], f32)
            nc.sync.dma_start(ident, id_d[:, :])
            l16 = consts.tile([G, P], f32)
            nc.sync.dma_start(l16, l16_d[:, :])
            indn = consts.tile([G, NB], f32)
            nc.sync.dma_start(indn, in_d[:, :])
            ones_r = consts.tile([1, P], f32)
            nc.sync.dma_start(ones_r, on_d[:, :])
            ones_h = consts.tile([1, P], f16)
            nc.sync.dma_start(ones_h, onh_d[:, :])
            eps_t = consts.tile([1, 1], f32)
            nc.vector.memset(eps_t, LN_EPS)

            # Sem-absorption ops: walrus allows very few sync waits per
            # compute instruction, so let each engine observe the const-load
            # DMA sems via tiny reads up front, keeping the hot-loop
            # instructions at <=1 wait each.
            cs4r = consts.tile([P, C], f32)
            nc.vector.memset(cs4r, 0.0)
            ab_ac = smp.tile([1, 1], f32, tag="ab_ac")
            nc.scalar.copy(ab_ac, bb[0:1, 0:1])
            nc.scalar.copy(ab_ac, eps_t[0:1, 0:1])
            ab_dv = smp.tile([1, 1], f32, tag="ab_dv")
            nc.vector.tensor_copy(ab_dv, msel4[0:1, 0:1])
            ab_te = mps.tile([1, 1], f32, tag="m")
            nc.tensor.matmul(ab_te, ones_h[:, 0:1], ones_h[:, 0:1],
                             start=True, stop=True)

            states = {}

            def emit_attn_chunk(s, ck):
                if ck == 0:
                    logits = lgp.tile([P, NT, G], f32, tag="logits")
                    esb = lgp.tile([P, NT, G], f16, tag="esb")
                    ctx_ps = ctxps.tile([P, C], f32, tag="ctx")
                    states[s] = (logits, esb, ctx_ps, [])
                logits, esb, ctx_ps, xhtiles = states[s]
                base = s * N
                t0, kch, hw = PLANS[s][ck]
                if s == 0 and ck < len(early_xh):
                    xh = early_xh[ck]
                else:
                    xh = xhp.tile([P, kch, C], f16, tag="xh")
                    rows = xf[base + t0 * P : base + (t0 + kch) * P, :]
                    # SWDGE cast-DMA: f32 HBM -> f16 SBUF at line rate;
                    # each partition line is one contiguous read.
                    nc.gpsimd.dma_start(
                        xh, rows.rearrange("(p k) c -> p k c", k=kch)
                    )
                xhtiles.append(xh)

                xw = xwp.tile([P, kch, C], f16, tag="xw")
                nc.vector.tensor_mul(xw, xh, w_rep[:, :kch, :])
                xwv = xw.rearrange("p k (g s) -> p k g s", s=SHI)
                x1 = xsp.tile([P, kch, G, SHI // 2], f16, tag="x1")
                nc.vector.tensor_add(
                    x1, xwv[:, :, :, : SHI // 2], xwv[:, :, :, SHI // 2 :]
                )
                x2 = xsp.tile([P, kch, G, SHI // 4], f16, tag="x2")
                nc.vector.tensor_add(
                    x2, x1[:, :, :, : SHI // 4], x1[:, :, :, SHI // 4 :]
                )
                nc.vector.reduce_sum(logits[:, t0 : t0 + kch, :], x2, AX)

                # E = exp((dot + b_mask) / 8); |logits| < ~1 so no
                # max-subtraction is needed for stability.
                nc.scalar.activation(
                    esb[:, t0 : t0 + kch, :],
                    logits[:, t0 : t0 + kch, :],
                    AF.Exp,
                    bias=bb[:, 0:1],
                    scale=0.125,
                )

                # ctx matmuls, col-tiled: strip j = t % 4 accumulates
                # into ctx_ps[32j:32j+8, :]; the 4 strips use distinct
                # 32-col PE groups and run concurrently.
                for t in range(t0, t0 + kch):
                    j = t % NSTRIP
                    nc.tensor.matmul(
                        ctx_ps[32 * j : 32 * j + G, :],
                        esb[:, t, :],
                        xh[:, t - t0, :],
                        start=(t == j),
                        stop=(t >= NT - NSTRIP),
                        tile_position=(0, 32 * j),
                        skip_group_check=True,
                    )

            def tail_chain(s, res):
                """softmax denom + ctx extract + MLP + t broadcast, as a
                generator so its steps can be woven between other phases'
                chunks (each step's cross-engine deps then resolve during
                the neighbouring chunk's work instead of stalling DVE)."""
                logits, esb, ctx_ps, xhtiles = states[s]

                # ---- softmax denominator: S[g] = sum_n E[n, g]
                sp = smp.tile([P, G], f32, tag="sp")
                nc.vector.reduce_sum(sp, esb.rearrange("p t g -> p g t"), AX)
                for j in range(NSTRIP):
                    nc.scalar.copy(
                        cs4r[32 * j : 32 * j + G, :],
                        ctx_ps[32 * j : 32 * j + G, :],
                    )
                yield
                spt = mps.tile([G, P], f32, tag="m")
                nc.tensor.transpose(spt, sp, ident)
                ssum = smp.tile([G, 1], f32, tag="ssum")
                nc.vector.reduce_sum(ssum, spt, AX)
                sinv = smp.tile([G, 1], f32, tag="sinv")
                nc.vector.reciprocal(sinv, ssum)
                yield
                # ---- 1/S as a per-partition column: sinv_pn[p, cb] =
                # sinv[2*cb + p//64], built as l16.T @ (indn * sinv).
                rsc = smp.tile([G, NB], f32, tag="rsc")
                nc.vector.tensor_scalar(
                    out=rsc, in0=indn, scalar1=sinv, scalar2=None, op0=MUL
                )
                spn_ps = mps.tile([P, NB], f32, tag="m")
                nc.tensor.matmul(spn_ps, l16, rsc, start=True, stop=True)
                sinv_pn = smp.tile([P, NB], f32, tag="sinv_pn")
                nc.vector.tensor_copy(sinv_pn, spn_ps)
                yield
                # ---- ctx extract: strips were copied out of PSUM into the
                # zeroed cs4r; mask all strips in one DVE op, then transpose
                # each 128-channel block and reduce (non-strip rows zero).
                cs4 = smp.tile([P, C], f32, tag="cs4")
                nc.vector.tensor_mul(cs4, cs4r, msel4)
                yield
                ctxt = smp.tile([P, NB], f32, tag="ctxt")
                for cb in range(NB):
                    tp4 = mps.tile([P, P], f32, tag="m")
                    nc.tensor.transpose(
                        tp4, cs4[:, cb * P : (cb + 1) * P], ident
                    )
                    nc.vector.reduce_sum(ctxt[:, cb : cb + 1], tp4, AX)
                    if cb == 1:
                        yield
                ctxs16 = smp.tile([P, NB], f16, tag="ctxs16")
                nc.vector.tensor_mul(ctxs16, ctxt, sinv_pn)
                yield
                # ---- h = ctx @ w1 + b1
                h_ps = mps.tile([1, C], f32, tag="m")
                for j in range(NB):
                    nc.tensor.matmul(
                        h_ps,
                        ctxs16[:, j : j + 1],
                        w1s[:, j, :],
                        start=(j == 0), stop=False,
                    )
                nc.tensor.matmul(
                    h_ps, ones_h[:, 0:1], b1s, start=False, stop=True,
                )
                yield
                # ---- LayerNorm over C, then ReLU
                musum = smp.tile([1, 1], f32, tag="musum")
                nc.vector.reduce_sum(musum, h_ps, AX)
                mu = smp.tile([1, 1], f32, tag="mu")
                nc.vector.tensor_scalar_mul(mu, musum, 1.0 / C)
                hc = smp.tile([1, C], f32, tag="hc")
                nc.vector.tensor_scalar(
                    out=hc, in0=h_ps, scalar1=mu, scalar2=None, op0=SUB
                )
                sq = smp.tile([1, C], f32, tag="sq")
                varsum = smp.tile([1, 1], f32, tag="varsum")
                nc.scalar.activation(sq, hc, AF.Square, accum_out=varsum)
                yield
                std = smp.tile([1, 1], f32, tag="std")
                nc.scalar.activation(
                    std, varsum, AF.Sqrt, bias=eps_t[:, 0:1], scale=1.0 / C
                )
                rstd = smp.tile([1, 1], f32, tag="rstd")
                nc.vector.reciprocal(rstd, std)
                hn = smp.tile([1, C], f32, tag="hn")
                nc.vector.scalar_tensor_tensor(
                    out=hn, in0=hc, scalar=rstd, in1=gms, op0=MUL, op1=MUL
                )
                hb = smp.tile([1, C], f32, tag="hb")
                nc.vector.tensor_add(hb, hn, bts)
                rl = smp.tile([1, C], f32, tag="rl")
                nc.scalar.activation(rl, hb, AF.Relu)
                yield
                # ---- t = relu_h @ w2 + b2 (transpose relu_h to [128, 4])
                rt_ps = mps.tile([P, NB], f32, tag="m")
                for j in range(NB):
                    nc.tensor.transpose(
                        rt_ps[:, j : j + 1],
                        rl[:, j * P : (j + 1) * P],
                        ones_r[:, 0:1],
                    )
                rts = smp.tile([P, NB], f16, tag="rts")
                nc.vector.tensor_copy(rts, rt_ps)
                yield
                t_ps = mps.tile([1, C], f32, tag="m")
                for j in range(NB):
                    nc.tensor.matmul(
                        t_ps,
                        rts[:, j : j + 1],
                        w2s[:, j, :],
                        start=(j == 0), stop=False,
                    )
                nc.tensor.matmul(
                    t_ps, ones_h[:, 0:1], b2s, start=False, stop=True,
                )
                tsb16 = smp.tile([1, C], f16, tag="tsb16")
                nc.scalar.copy(tsb16, t_ps)
                yield
                # ---- broadcast t to all partitions, fp16
                trep_ps = rps.tile([P, C], f32, tag="rp")
                nc.tensor.matmul(trep_ps, ones_h, tsb16, start=True, stop=True)
                trep = trp.tile([P, KCH, C], f16, tag="trep")
                for k in range(KCH):
                    nc.scalar.copy(trep[:, k, :], trep_ps)
                res["trep"] = trep

            def emit_resid_chunk(s, ck, trep):
                # residual add out = x + t, fp16, in place on xh
                xhtiles = states[s][3]
                xh = xhtiles[ck]
                base = s * N
                t0, kch, _ = PLANS[s][ck]
                nc.vector.tensor_add(xh, xh, trep[:, :kch, :])
                rows = of[base + t0 * P : base + (t0 + kch) * P, :]
                nc.sync.dma_start(
                    rows.rearrange("(p k) c -> p k c", k=kch), xh
                )

            def drive(gen):
                try:
                    next(gen)
                    return True
                except StopIteration:
                    return False

            # ---- interleaved emission: sample 0's tail chain is woven
            # into sample 1's attention chunks; sample 1's tail chain into
            # sample 0's residual/store chunks.  Each engine's stream then
            # alternates chain steps with bulk work, so the chain's
            # cross-engine latency hides behind the bulk ops.
            for ck in range(len(PLAN0)):
                emit_attn_chunk(0, ck)
            res0, res1 = {}, {}
            # chain0 woven into sample 1's first attention chunks; then
            # sample 0's residual adds+stores interleave with the rest of
            # sample 1's attention so the output stream starts early.
            g0 = tail_chain(0, res0)
            for ck in range(len(PLAN1)):
                emit_attn_chunk(1, ck)
                drive(g0)
                drive(g0)
            while drive(g0):
                pass
            g1 = tail_chain(1, res1)
            for ck in range(len(PLAN0)):
                emit_resid_chunk(0, ck, res0["trep"])
                drive(g1)
                drive(g1)
            while drive(g1):
                pass
            for ck in range(len(PLAN1)):
                emit_resid_chunk(1, ck, res1["trep"])

    nc.finalize()
    return nc


def _prep_shared(inputs):
    w_mask = np.asarray(inputs["w_mask"], np.float32).reshape(SHI)
    b_mask = np.asarray(inputs["b_mask"], np.float32).reshape(1)
    w1 = np.asarray(inputs["w1"], np.float32)
    w2 = np.asarray(inputs["w2"], np.float32)

    # strip-expanded header mask: rows {32*j + g} hold header g's channel
    # mask (ctx strip j lands on PSUM partitions 32*j..32*j+7)
    msel4 = np.zeros((P, C), np.float32)
    gsel = ((np.arange(C)[None, :] // SHI) == np.arange(G)[:, None]).astype(
        np.float32
    )
    for j in range(NSTRIP):
        msel4[32 * j : 32 * j + G, :] = gsel
    # l16[g, p] = (g%2 == p//64); indn[g, cb] = (g//2 == cb):
    # (l16.T @ (indn * sinv))[p, cb] = sinv[2*cb + p//64]
    l16 = (np.arange(G)[:, None] % 2 == np.arange(P)[None, :] // 64).astype(
        np.float32
    )
    indn = (np.arange(G)[:, None] // 2 == np.arange(NB)[None, :]).astype(
        np.float32
    )

    shared = {
        "w_rep": np.broadcast_to(np.tile(w_mask, G), (P, KCH, C)).astype(
            np.float16
        ),
        "b_bias": np.full((P, 1), b_mask[0] * 0.125, np.float32),
        "w1p": np.ascontiguousarray(
            w1.reshape(NB, P, C).transpose(1, 0, 2)
        ).astype(np.float16),
        "w2p": np.ascontiguousarray(
            w2.reshape(NB, P, C).transpose(1, 0, 2)
        ).astype(np.float16),
        "b1r": np.asarray(inputs["b1"], np.float16).reshape(1, C),
        "b2r": np.asarray(inputs["b2"], np.float16).reshape(1, C),
        "gammar": np.asarray(inputs["gamma"], np.float32).reshape(1, C),
        "betar": np.asarray(inputs["beta"], np.float32).reshape(1, C),
        "mask_sel4": msel4,
        "ident": np.eye(P, dtype=np.float32),
        "l16": l16,
        "indn": indn,
        "ones_row": np.ones((1, P), np.float32),
        "ones_h": np.ones((1, P), np.float16),
    }
    return shared


def make_in_maps(inputs):
    x = np.asarray(inputs["x"], np.float32)
    shared = _prep_shared(inputs)
    in_maps = []
    for i in range(NCORES):
        m = dict(shared)
        m["x"] = np.ascontiguousarray(x[i * BPC : (i + 1) * BPC])
        in_maps.append(m)
    return in_maps


def _axon_device_reset():
    """Clear any wedged NRT exec-unit state left by a previous session."""
    try:
        import ctypes

        import jax

        jax.devices()
        lib = ctypes.CDLL("/opt/axon/libaxon_pjrt.so")
        lib.axon_reset.restype = ctypes.c_int64
        lib.axon_reset()
    except Exception:
        pass


def kernel(**inputs):
    from concourse.bass_utils import run_bass_kernel_spmd

    _axon_device_reset()
    nc = build_nc()
    in_maps = make_in_maps(inputs)
    res = run_bass_kernel_spmd(nc, in_maps, list(range(NCORES)))
    out = np.concatenate([r["out"] for r in res.results], axis=0)
    return out.astype(np.float32)
